# revision 1
# baseline (speedup 1.0000x reference)
"""Trainium2 Bass kernel for the 2-hop GNN (GCN + SAGE + BatchNorm) problem.

Strategy (8 NeuronCores, SPMD single program):
  - Destination (user-node) sharding: core k owns output rows [k*12500, (k+1)*12500).
  - Host prep is integer index manipulation only (sort/bucket edges by
    destination, pad, build gather-index tables). All FP math runs on device.
  - Scatter-add (segment_sum) via sorted-window one-hot matmuls: edges are
    bucketed into 64-destination windows; per 128-edge block build
    S[e,d] = (col_in_window[e]==d) * dis[row[e]] on VectorE, then
    PSUM-accumulate vals^T @ S on TensorE, giving feature-major aggregates
    agg_uT [85, 12544] / agg_cT [64, 12544] resident in SBUF.
  - Row gathers (userF rows, comment rows, dis scalars) from DRAM via gpsimd
    indirect DMA, chunked.
  - userF built on device: newF = [u_feature|1] @ M' (M' assembled from
    e-tables via selector matmuls) plus an embedding-table gather.
  - GCN degree / SAGE counts via one-hot count matmuls; dis AllGathered
    across cores; BatchNorm stats AllReduced.
"""

import numpy as np

import concourse.bass as bass
import concourse.bacc as bacc
import concourse.tile as tile
import concourse.mybir as mybir
from concourse import bass_utils
from concourse.bass import IndirectOffsetOnAxis

F32 = mybir.dt.float32
I32 = mybir.dt.int32

# Problem constants
U1 = 50000
U2 = 50000
U = 100000
C = 200000
E = 1000000
ED = 85
DC = 64
H = 128
NH = 2
# newF layout: em0[0:16], uf1,uf2[16:18], em3[18:34], uf4,uf5,uf6[34:37],
#              em7[37:53], em8[53:69], em9[69:85]
EMB_BLOCKS = [(0, 0), (3, 18), (7, 37), (8, 53), (9, 69)]  # (src_col, out_lo)
PASS_COLS = [(1, 16), (2, 17), (4, 34), (5, 35), (6, 36)]  # (src_col, out_col)

NCORES = 8
L = 12500            # real output rows per core
WIN = 64             # destinations per window
NW = 196             # windows per core (196*64 = 12544)
LP = NW * WIN        # padded local rows = 12544
NLOC = LP // 128     # 98 local row-tiles
NFT = (U1 + 127) // 128   # 391 newF row-tiles
NFROWS = NFT * 128        # 50048
UFPAD = U1 + NFROWS       # userF rows incl. build overhang = 100096
FTILE = 512               # final-pass row-tile
NFTILES = (LP + FTILE - 1) // FTILE  # 25 (24x512 + 1x256)


def _shard_rel(row, col, with_dis_idx):
    """Bucket edges (row->col) by destination core and 64-dest window."""
    order = np.argsort(col, kind="stable")
    row_s = row[order]
    col_s = col[order]
    shard = col_s // L
    w_in = (col_s % L) // WIN
    cw = (col_s % L) % WIN
    key = shard * NW + w_in
    counts = np.bincount(key, minlength=NCORES * NW)
    B = int(np.ceil(counts.max() / 128.0))
    NBLK = NW * B
    starts = np.zeros(NCORES * NW, dtype=np.int64)
    np.cumsum(counts[:-1], out=starts[1:])
    pos = np.arange(len(col_s)) - starts[key]
    b = pos // 128
    p = pos % 128
    j = w_in * B + b
    gidx = np.zeros((NCORES, 128, NBLK), np.int32)
    colw = np.full((NCORES, 128, NBLK), -1.0, np.float32)
    gidx[shard, p, j] = row_s
    colw[shard, p, j] = cw.astype(np.float32)
    out = {"gidx": gidx, "colw": colw, "B": B, "NBLK": NBLK}
    if with_dis_idx:
        disidx = np.zeros((NCORES, 128, NBLK), np.int32)
        dpos = (row_s // L) * LP + (row_s % L)
        disidx[shard, p, j] = dpos.astype(np.int32)
        out["disidx"] = disidx
    return out


def host_prep(inputs):
    no_Nidx = np.asarray(inputs["no_Nidx"]).astype(np.int64)
    u_feature = np.asarray(inputs["u_feature"], dtype=np.float32)
    comment_x = np.asarray(inputs["comment_x"], dtype=np.float32)
    edge_uu = np.asarray(inputs["edge_uu"]).astype(np.int64)
    cu_src = np.asarray(inputs["edge_cu_src"]).astype(np.int64)
    cu_dst = np.asarray(inputs["edge_cu_dst"]).astype(np.int64)

    uu = _shard_rel(edge_uu[0], edge_uu[1], with_dis_idx=True)
    cu = _shard_rel(cu_src, cu_dst, with_dis_idx=False)

    ufp = np.zeros((NFROWS, 10), np.float32)
    ufp[:U1] = u_feature

    nop = np.zeros(NFROWS, np.int64)
    nop[:U2] = no_Nidx
    no_N_t = np.ascontiguousarray(nop.reshape(NFT, 128).T).astype(np.int32)

    loc = np.zeros((NCORES, 128, NLOC), np.int32)
    for k in range(NCORES):
        v = np.arange(LP, dtype=np.int64) + k * L
        v[L:] = 0  # pad rows -> row 0 (finite data; results discarded)
        loc[k] = v.reshape(NLOC, 128).T

    bsel = np.zeros((5, 2, 11), np.float32)
    for i, (src, _lo) in enumerate(EMB_BLOCKS):
        bsel[i, 0, src] = -1.0
        bsel[i, 1, src] = 1.0
        bsel[i, 0, 10] = 1.0
    kmat = np.zeros((11, ED), np.float32)
    for src, oc in PASS_COLS:
        kmat[src, oc] = 1.0
    iota64 = np.tile(np.arange(WIN, dtype=np.float32), (128, 1))
    ident = np.eye(128, dtype=np.float32)

    shared = {
        "u_feature": ufp,
        "no_N_t": no_N_t,
        "emb_table": np.asarray(inputs["emb_table"], dtype=np.float32),
        "comment_x": np.asarray(comment_x, dtype=np.float32),
        "e_tabs": np.stack(
            [np.asarray(inputs[n], dtype=np.float32) for n in ("e0", "e3", "e7", "e8", "e9")]
        ),
        "gcn_w": np.asarray(inputs["gcn_w"], dtype=np.float32),
        "gcn_b": np.asarray(inputs["gcn_b"], dtype=np.float32),
        "sage_l_w": np.asarray(inputs["sage_l_w"], dtype=np.float32),
        "sage_l_b": np.asarray(inputs["sage_l_b"], dtype=np.float32),
        "sage_r_w": np.asarray(inputs["sage_r_w"], dtype=np.float32),
        "bn_gamma": np.asarray(inputs["bn_gamma"], dtype=np.float32),
        "bn_beta": np.asarray(inputs["bn_beta"], dtype=np.float32),
        "bsel": bsel,
        "kmat": kmat,
        "iota64": iota64,
        "ident": ident,
    }
    percore = []
    for k in range(NCORES):
        m = dict(shared)
        m["uu_gidx"] = uu["gidx"][k]
        m["uu_colw"] = uu["colw"][k]
        m["uu_disidx"] = uu["disidx"][k]
        m["cu_gidx"] = cu["gidx"][k]
        m["cu_colw"] = cu["colw"][k]
        m["loc_idx"] = loc[k]
        percore.append(m)
    cfg = {"B_u": uu["B"], "NBLK_u": uu["NBLK"], "B_c": cu["B"], "NBLK_c": cu["NBLK"]}
    return percore, cfg


INPUT_SPECS = [
    ("u_feature", (NFROWS, 10), F32),
    ("no_N_t", (128, NFT), I32),
    ("emb_table", (U2, ED), F32),
    ("comment_x", (C, DC), F32),
    ("e_tabs", (5, 2, 16), F32),
    ("gcn_w", (NH, ED, H), F32),
    ("gcn_b", (NH, H), F32),
    ("sage_l_w", (NH, DC, H), F32),
    ("sage_l_b", (NH, H), F32),
    ("sage_r_w", (NH, ED, H), F32),
    ("bn_gamma", (H,), F32),
    ("bn_beta", (H,), F32),
    ("bsel", (5, 2, 11), F32),
    ("kmat", (11, ED), F32),
    ("iota64", (128, WIN), F32),
    ("ident", (128, 128), F32),
]


def build(nc, tc, io, out_ap, cfg):
    """Emit the kernel body. io: dict name->AP (DRAM inputs), out_ap [LP, H]."""
    B_u, NBLK_u = cfg["B_u"], cfg["NBLK_u"]
    B_c, NBLK_c = cfg["B_c"], cfg["NBLK_c"]
    AT = mybir.AluOpType
    AF = mybir.ActivationFunctionType
    AX = mybir.AxisListType
    RG = [list(range(NCORES))]

    # internal DRAM
    userF = nc.dram_tensor("userF_d", [UFPAD, ED], F32).ap()
    dis_shard = nc.dram_tensor("dis_shard_d", [LP], F32).ap()
    dis_full = nc.dram_tensor("dis_full_d", [NCORES * LP], F32, addr_space="Shared").ap()
    cnt_shard = nc.dram_tensor("cnt_shard_d", [LP], F32).ap()
    bn_in = nc.dram_tensor("bn_in_d", [H, 2], F32).ap()
    bn_out = nc.dram_tensor("bn_out_d", [H, 2], F32, addr_space="Shared").ap()

    import contextlib

    stack = contextlib.ExitStack()
    big = stack.enter_context(tc.tile_pool(name="big", bufs=1))
    agg_u = big.tile([ED, LP], F32, tag="agg_u")
    agg_c = big.tile([DC, LP], F32, tag="agg_c")
    deg_loc = big.tile([WIN, NW], F32, tag="deg_loc")
    dis_loc = big.tile([WIN, NW], F32, tag="dis_loc")
    cnt_loc = big.tile([WIN, NW], F32, tag="cnt_loc")
    iota_sb = big.tile([128, WIN], F32, tag="iota_sb")
    ident_sb = big.tile([128, 128], F32, tag="ident_sb")
    ones_col = big.tile([128, 1], F32, tag="ones_col")
    ones_1 = big.tile([1, 128], F32, tag="ones_1")
    no_N_sb = big.tile([128, NFT], I32, tag="no_N_sb")
    loc_idx_sb = big.tile([128, NLOC], I32, tag="loc_idx")
    mp_sb = big.tile([11, ED], F32, tag="mp_sb")
    wg_sb = [big.tile([ED, H], F32, name=f"wg{h}", tag=f"wg{h}") for h in range(NH)]
    wr_sb = [big.tile([ED, H], F32, name=f"wr{h}", tag=f"wr{h}") for h in range(NH)]
    wl_sb = [big.tile([DC, H], F32, name=f"wl{h}", tag=f"wl{h}") for h in range(NH)]
    bh_sb = [big.tile([H, 1], F32, name=f"bh{h}", tag=f"bh{h}") for h in range(NH)]
    nbh_sb = [big.tile([H, 1], F32, name=f"nbh{h}", tag=f"nbh{h}") for h in range(NH)]
    gam_sb = big.tile([H, 1], F32, tag="gam")
    bet_sb = big.tile([H, 1], F32, tag="bet")
    s_part = big.tile([H, NFTILES], F32, tag="s_part")
    sq_part = big.tile([H, NFTILES], F32, tag="sq_part")

    nc.sync.dma_start(out=iota_sb[:], in_=io["iota64"])
    nc.sync.dma_start(out=ident_sb[:], in_=io["ident"])
    nc.vector.memset(ones_col[:], 1.0)
    nc.vector.memset(ones_1[:], 1.0)
    nc.sync.dma_start(out=no_N_sb[:], in_=io["no_N_t"])
    nc.sync.dma_start(out=loc_idx_sb[:], in_=io["loc_idx"])
    for h in range(NH):
        nc.sync.dma_start(out=wg_sb[h][:], in_=io["gcn_w"][h])
        nc.sync.dma_start(out=wr_sb[h][:], in_=io["sage_r_w"][h])
        nc.sync.dma_start(out=wl_sb[h][:], in_=io["sage_l_w"][h])
    nc.sync.dma_start(out=gam_sb[:], in_=io["bn_gamma"][:, None])
    nc.sync.dma_start(out=bet_sb[:], in_=io["bn_beta"][:, None])

    # ---- M' (newF = [u_f | 1] @ M') and fused biases -------------------
    with (
        tc.tile_pool(name="bld", bufs=2) as bld,
        tc.tile_pool(name="bldp", bufs=2, space="PSUM") as bldp,
    ):
        nc.sync.dma_start(out=mp_sb[:], in_=io["kmat"])
        for i, (_src, lo) in enumerate(EMB_BLOCKS):
            e_sb = bld.tile([2, 16], F32, tag="e_sb")
            b_sb = bld.tile([2, 11], F32, tag="b_sb")
            nc.sync.dma_start(out=e_sb[:], in_=io["e_tabs"][i])
            nc.sync.dma_start(out=b_sb[:], in_=io["bsel"][i])
            mpp = bldp.tile([11, 16], F32, tag="mpp")
            nc.tensor.matmul(out=mpp[:], lhsT=b_sb[:], rhs=e_sb[:], start=True, stop=True)
            nc.vector.tensor_copy(out=mp_sb[:, lo : lo + 16], in_=mpp[:])
        for h in range(NH):
            t1 = bld.tile([H, 1], F32, tag="t1")
            t2 = bld.tile([H, 1], F32, tag="t2")
            nc.sync.dma_start(out=t1[:], in_=io["gcn_b"][h][:, None])
            nc.sync.dma_start(out=t2[:], in_=io["sage_l_b"][h][:, None])
            nc.vector.tensor_tensor(out=bh_sb[h][:], in0=t1[:], in1=t2[:], op=AT.add)
            nc.vector.tensor_scalar(
                out=nbh_sb[h][:], in0=bh_sb[h][:], scalar1=-1.0, scalar2=None, op0=AT.mult
            )

    # ---- build userF in DRAM (every core, redundantly) -----------------
    with (
        tc.tile_pool(name="ufb", bufs=3) as ufb,
        tc.tile_pool(name="ufbp", bufs=3, space="PSUM") as ufbp,
    ):
        ufr = io["u_feature"].rearrange("(n p) c -> n p c", p=128)
        for n in range(NFT):
            uft = ufb.tile([128, 11], F32, tag="uft")
            nc.sync.dma_start(out=uft[:, :10], in_=ufr[n])
            nc.vector.memset(uft[:, 10:11], 1.0)
            tp = ufbp.tile([11, 128], F32, tag="tp")
            nc.tensor.transpose(out=tp[:], in_=uft[:], identity=ident_sb[:])
            ufT = ufb.tile([11, 128], F32, tag="ufT")
            nc.scalar.activation(out=ufT[:], in_=tp[:], func=AF.Copy)
            nfp = ufbp.tile([128, ED], F32, tag="nfp")
            nc.tensor.matmul(out=nfp[:], lhsT=ufT[:], rhs=mp_sb[:], start=True, stop=True)
            nf = ufb.tile([128, ED], F32, tag="nf")
            nc.scalar.activation(out=nf[:], in_=nfp[:], func=AF.Copy)
            rows = 128 if (n + 1) * 128 <= U1 else U1 - n * 128
            nc.sync.dma_start(out=userF[n * 128 : n * 128 + rows, :], in_=nf[:rows, :])
        # embedding half: gather emb_table rows by no_Nidx -> userF[U1:]
        for j in range(NFT):
            st = ufb.tile([128, ED], F32, tag="embst")
            nc.gpsimd.indirect_dma_start(
                out=st[:],
                out_offset=None,
                in_=io["emb_table"],
                in_offset=IndirectOffsetOnAxis(ap=no_N_sb[:, j : j + 1], axis=0),
            )
            nc.sync.dma_start(
                out=userF[U1 + j * 128 : U1 + (j + 1) * 128, :], in_=st[:]
            )

    # ---- phase A: cu main pass (agg_c, cnt) + uu deg pass --------------
    ecu = io["comment_x"]
    with (
        tc.tile_pool(name="pha", bufs=2) as pha,
        tc.tile_pool(name="phas", bufs=B_c + 2) as phas,
        tc.tile_pool(name="phap", bufs=2, space="PSUM") as phap,
        tc.tile_pool(name="phad", bufs=1) as phad,
    ):
        cu_colw_sb = phad.tile([128, NBLK_c], F32, tag="cu_colw")
        cu_gidx_sb = phad.tile([128, NBLK_c], I32, tag="cu_gidx")
        uu_colw_a = phad.tile([128, NBLK_u], F32, tag="uu_colw_a")
        nc.sync.dma_start(out=cu_colw_sb[:], in_=io["cu_colw"])
        nc.sync.dma_start(out=cu_gidx_sb[:], in_=io["cu_gidx"])
        nc.sync.dma_start(out=uu_colw_a[:], in_=io["uu_colw"])
        for w in range(NW):
            pc = phap.tile([DC, WIN], F32, tag="pc")
            pcnt = phap.tile([WIN, 1], F32, tag="pcnt")
            s_tiles = []
            for b in range(B_c):
                j = w * B_c + b
                st = pha.tile([128, DC], F32, tag="cust", bufs=4)
                nc.gpsimd.indirect_dma_start(
                    out=st[:],
                    out_offset=None,
                    in_=ecu,
                    in_offset=IndirectOffsetOnAxis(
                        ap=cu_gidx_sb[:, j : j + 1], axis=0
                    ),
                )
                S = phas.tile([128, WIN], F32, tag="S")
                nc.vector.tensor_scalar(
                    out=S[:],
                    in0=iota_sb[:],
                    scalar1=cu_colw_sb[:, j : j + 1],
                    scalar2=None,
                    op0=AT.is_equal,
                )
                s_tiles.append(S)
                nc.tensor.matmul(
                    out=pc[:],
                    lhsT=st[:],
                    rhs=S[:],
                    start=(b == 0),
                    stop=(b == B_c - 1),
                )
            for b in range(B_c):
                nc.tensor.matmul(
                    out=pcnt[:],
                    lhsT=s_tiles[b][:],
                    rhs=ones_col[:],
                    start=(b == 0),
                    stop=(b == B_c - 1),
                )
            nc.scalar.activation(
                out=agg_c[:, w * WIN : (w + 1) * WIN], in_=pc[:], func=AF.Copy
            )
            nc.scalar.activation(out=cnt_loc[:, w : w + 1], in_=pcnt[:], func=AF.Copy)
        # uu deg pass (no gather needed)
        for w in range(NW):
            pdeg = phap.tile([WIN, 1], F32, tag="pdeg")
            for b in range(B_u):
                j = w * B_u + b
                S = phas.tile([128, WIN], F32, tag="S")
                nc.vector.tensor_scalar(
                    out=S[:],
                    in0=iota_sb[:],
                    scalar1=uu_colw_a[:, j : j + 1],
                    scalar2=None,
                    op0=AT.is_equal,
                )
                nc.tensor.matmul(
                    out=pdeg[:],
                    lhsT=S[:],
                    rhs=ones_col[:],
                    start=(b == 0),
                    stop=(b == B_u - 1),
                )
            nc.scalar.activation(out=deg_loc[:, w : w + 1], in_=pdeg[:], func=AF.Copy)

    # ---- phase B: dis = (deg>0)*rsqrt(max(deg,1)); allgather -----------
    uupool_cm = tc.tile_pool(name="uud", bufs=1)
    uupool = uupool_cm.__enter__()
    uu_colw_sb = uupool.tile([128, NBLK_u], F32, tag="uu_colw")
    uu_gidx_sb = uupool.tile([128, NBLK_u], I32, tag="uu_gidx")
    uu_disidx_sb = uupool.tile([128, NBLK_u], I32, tag="uu_disidx")
    uu_disrow_sb = uupool.tile([128, NBLK_u], F32, tag="uu_disrow")
    nc.sync.dma_start(out=uu_colw_sb[:], in_=io["uu_colw"])
    nc.sync.dma_start(out=uu_gidx_sb[:], in_=io["uu_gidx"])
    nc.sync.dma_start(out=uu_disidx_sb[:], in_=io["uu_disidx"])
    with tc.tile_pool(name="phb", bufs=1) as phb:
        mx = phb.tile([WIN, NW], F32, tag="mx")
        nc.vector.tensor_scalar(
            out=mx[:], in0=deg_loc[:], scalar1=1.0, scalar2=None, op0=AT.max
        )
        rc = phb.tile([WIN, NW], F32, tag="rc")
        nc.vector.reciprocal(out=rc[:], in_=mx[:])
        sq = phb.tile([WIN, NW], F32, tag="sq")
        nc.scalar.activation(out=sq[:], in_=rc[:], func=AF.Sqrt)
        mk = phb.tile([WIN, NW], F32, tag="mk")
        nc.vector.tensor_scalar(
            out=mk[:], in0=deg_loc[:], scalar1=0.0, scalar2=None, op0=AT.is_gt
        )
        nc.vector.tensor_tensor(out=dis_loc[:], in0=sq[:], in1=mk[:], op=AT.mult)
        # cntinv
        cmx = phb.tile([WIN, NW], F32, tag="cmx")
        nc.vector.tensor_scalar(
            out=cmx[:], in0=cnt_loc[:], scalar1=1.0, scalar2=None, op0=AT.max
        )
        cinv = phb.tile([WIN, NW], F32, tag="cinv")
        nc.vector.reciprocal(out=cinv[:], in_=cmx[:])
        # to DRAM shards (elementwise transposed APs)
        nc.sync.dma_start(
            out=dis_shard.rearrange("(w p) -> p w", p=WIN), in_=dis_loc[:]
        )
        nc.sync.dma_start(
            out=cnt_shard.rearrange("(w p) -> p w", p=WIN), in_=cinv[:]
        )
        nc.gpsimd.collective_compute(
            "AllGather",
            mybir.AluOpType.bypass,
            replica_groups=RG,
            ins=[dis_shard],
            outs=[dis_full],
        )
        # per-edge dis[row] gather (row-major positions precomputed on host)
        for j in range(NBLK_u):
            nc.gpsimd.indirect_dma_start(
                out=uu_disrow_sb[:, j : j + 1],
                out_offset=None,
                in_=dis_full[:, None],
                in_offset=IndirectOffsetOnAxis(ap=uu_disidx_sb[:, j : j + 1], axis=0),
            )

    if io.get("dbg_dis") is not None:
        nc.sync.dma_start(out=io["dbg_dis"], in_=dis_full)
        nc.sync.dma_start(out=io["dbg_disrow"], in_=uu_disrow_sb[:])

    # ---- phase C: uu main pass (agg_u) ---------------------------------
    with (
        tc.tile_pool(name="phc", bufs=2) as phc,
        tc.tile_pool(name="phcs", bufs=3) as phcs,
        tc.tile_pool(name="phcp", bufs=2, space="PSUM") as phcp,
    ):
        for w in range(NW):
            pu = phcp.tile([ED, WIN], F32, tag="pu")
            for b in range(B_u):
                j = w * B_u + b
                st = phc.tile([128, ED], F32, tag="uust", bufs=4)
                nc.gpsimd.indirect_dma_start(
                    out=st[:],
                    out_offset=None,
                    in_=userF,
                    in_offset=IndirectOffsetOnAxis(
                        ap=uu_gidx_sb[:, j : j + 1], axis=0
                    ),
                )
                S = phcs.tile([128, WIN], F32, tag="S2")
                nc.vector.tensor_scalar(
                    out=S[:],
                    in0=iota_sb[:],
                    scalar1=uu_colw_sb[:, j : j + 1],
                    scalar2=uu_disrow_sb[:, j : j + 1],
                    op0=AT.is_equal,
                    op1=AT.mult,
                )
                nc.tensor.matmul(
                    out=pu[:],
                    lhsT=st[:],
                    rhs=S[:],
                    start=(b == 0),
                    stop=(b == B_u - 1),
                )
            nc.scalar.activation(
                out=agg_u[:, w * WIN : (w + 1) * WIN], in_=pu[:], func=AF.Copy
            )

    uupool_cm.__exit__(None, None, None)

    # ---- scale agg_u by dis[col], agg_c by 1/cnt -----------------------
    with (
        tc.tile_pool(name="scl", bufs=2) as scl,
        tc.tile_pool(name="sclp", bufs=2, space="PSUM") as sclp,
    ):
        for t in range(NFTILES):
            t0 = t * FTILE
            tn = min(FTILE, LP - t0)
            dis_row = scl.tile([1, FTILE], F32, tag="dis_row")
            nc.sync.dma_start(out=dis_row[:, :tn], in_=dis_shard[None, t0 : t0 + tn])
            cnt_row = scl.tile([1, FTILE], F32, tag="cnt_row")
            nc.sync.dma_start(out=cnt_row[:, :tn], in_=cnt_shard[None, t0 : t0 + tn])
            pb = sclp.tile([128, FTILE], F32, tag="pb")
            nc.tensor.matmul(
                out=pb[:, :tn],
                lhsT=ones_1[:],
                rhs=dis_row[:, :tn],
                start=True,
                stop=True,
            )
            nc.vector.tensor_tensor(
                out=agg_u[:, t0 : t0 + tn],
                in0=agg_u[:, t0 : t0 + tn],
                in1=pb[:ED, :tn],
                op=AT.mult,
            )
            pb2 = sclp.tile([128, FTILE], F32, tag="pb2")
            nc.tensor.matmul(
                out=pb2[:, :tn],
                lhsT=ones_1[:],
                rhs=cnt_row[:, :tn],
                start=True,
                stop=True,
            )
            nc.vector.tensor_tensor(
                out=agg_c[:, t0 : t0 + tn],
                in0=agg_c[:, t0 : t0 + tn],
                in1=pb2[:DC, :tn],
                op=AT.mult,
            )

    if io.get("dbg_aggu") is not None:
        nc.sync.dma_start(out=io["dbg_aggu"], in_=agg_u[:, :512])
        nc.sync.dma_start(out=io["dbg_aggc"], in_=agg_c[:, :512])
        nc.sync.dma_start(out=io["dbg_ufa"], in_=userF[:512, :])
        nc.sync.dma_start(out=io["dbg_ufb"], in_=userF[U1 : U1 + 512, :])

    # ---- final: matmuls + leaky relu + hop sum + BN stats --------------
    nodep_cm = tc.tile_pool(name="nodep", bufs=1)
    nodep = nodep_cm.__enter__()
    node = nodep.tile([H, LP], F32, tag="node")
    with (
        tc.tile_pool(name="fin", bufs=2) as fin,
        tc.tile_pool(name="finp", bufs=4, space="PSUM") as finp,
    ):
        for t in range(NFTILES):
            t0 = t * FTILE
            tn = min(FTILE, LP - t0)
            nsub = tn // 128
            # userF_T for this tile via gather + PE transpose
            ufg = fin.tile([128, (FTILE // 128) * ED], F32, tag="ufg")
            for s in range(nsub):
                nc.gpsimd.indirect_dma_start(
                    out=ufg[:, s * ED : (s + 1) * ED],
                    out_offset=None,
                    in_=userF,
                    in_offset=IndirectOffsetOnAxis(
                        ap=loc_idx_sb[:, t0 // 128 + s : t0 // 128 + s + 1], axis=0
                    ),
                )
            ufT = fin.tile([ED, FTILE], F32, tag="ufT")
            for s in range(nsub):
                ptp = finp.tile([ED, 128], F32, tag="ptp")
                nc.tensor.transpose(
                    out=ptp[:],
                    in_=ufg[:, s * ED : (s + 1) * ED],
                    identity=ident_sb[:],
                )
                nc.scalar.activation(
                    out=ufT[:, s * 128 : (s + 1) * 128], in_=ptp[:], func=AF.Copy
                )
            rel = []
            for h in range(NH):
                ph = finp.tile([H, FTILE], F32, tag="ph")
                nc.tensor.matmul(
                    out=ph[:, :tn], lhsT=wg_sb[h][:], rhs=agg_u[:, t0 : t0 + tn],
                    start=True, stop=False,
                )
                nc.tensor.matmul(
                    out=ph[:, :tn], lhsT=wr_sb[h][:], rhs=ufT[:, :tn],
                    start=False, stop=False,
                )
                nc.tensor.matmul(
                    out=ph[:, :tn], lhsT=wl_sb[h][:], rhs=agg_c[:, t0 : t0 + tn],
                    start=False, stop=True,
                )
                # leaky_relu(y+b, 0.3) = relu(y+b) - 0.3*relu(-(y+b))
                rp = fin.tile([H, FTILE], F32, tag="rp")
                nc.scalar.activation(
                    out=rp[:, :tn], in_=ph[:, :tn], func=AF.Relu, bias=bh_sb[h][:]
                )
                rn = fin.tile([H, FTILE], F32, tag="rn")
                nc.scalar.activation(
                    out=rn[:, :tn], in_=ph[:, :tn], func=AF.Relu,
                    bias=nbh_sb[h][:], scale=-1.0,
                )
                rel.append((rp, rn))
            a1 = fin.tile([H, FTILE], F32, tag="a1")
            nc.vector.tensor_tensor(
                out=a1[:, :tn], in0=rel[0][0][:, :tn], in1=rel[1][0][:, :tn], op=AT.add
            )
            a2 = fin.tile([H, FTILE], F32, tag="a2")
            nc.vector.tensor_tensor(
                out=a2[:, :tn], in0=rel[0][1][:, :tn], in1=rel[1][1][:, :tn], op=AT.add
            )
            a3 = fin.tile([H, FTILE], F32, tag="a3")
            nc.vector.tensor_scalar(
                out=a3[:, :tn], in0=a2[:, :tn], scalar1=-0.3, scalar2=None, op0=AT.mult
            )
            nc.vector.tensor_tensor(
                out=node[:, t0 : t0 + tn], in0=a1[:, :tn], in1=a3[:, :tn], op=AT.add
            )
            if t < NFTILES - 1:
                nc.vector.tensor_reduce(
                    out=s_part[:, t : t + 1], in_=node[:, t0 : t0 + tn],
                    axis=AX.X, op=AT.add,
                )
                sqs = fin.tile([H, FTILE], F32, tag="sqs")
                nc.scalar.activation(
                    out=sqs[:, :tn], in_=node[:, t0 : t0 + tn], func=AF.Square,
                    accum_out=sq_part[:, t : t + 1],
                )
        # zero pad columns, then stats for the last tile
        nc.vector.memset(node[:, L:LP], 0.0)
        t = NFTILES - 1
        t0 = t * FTILE
        tn = LP - t0
        nc.vector.tensor_reduce(
            out=s_part[:, t : t + 1], in_=node[:, t0 : t0 + tn], axis=AX.X, op=AT.add
        )
        sqs = fin.tile([H, FTILE], F32, tag="sqs")
        nc.scalar.activation(
            out=sqs[:, :tn], in_=node[:, t0 : t0 + tn], func=AF.Square,
            accum_out=sq_part[:, t : t + 1],
        )

    # ---- BN: allreduce stats, normalize, transpose out -----------------
    with (
        tc.tile_pool(name="bn", bufs=2) as bn,
        tc.tile_pool(name="bnp", bufs=2, space="PSUM") as bnp,
    ):
        stat = bn.tile([H, 2], F32, tag="stat")
        nc.vector.tensor_reduce(
            out=stat[:, 0:1], in_=s_part[:], axis=AX.X, op=AT.add
        )
        nc.vector.tensor_reduce(
            out=stat[:, 1:2], in_=sq_part[:], axis=AX.X, op=AT.add
        )
        nc.sync.dma_start(out=bn_in, in_=stat[:])
        nc.gpsimd.collective_compute(
            "AllReduce",
            mybir.AluOpType.add,
            replica_groups=RG,
            ins=[bn_in],
            outs=[bn_out],
        )
        gstat = bn.tile([H, 2], F32, tag="gstat")
        nc.sync.dma_start(out=gstat[:], in_=bn_out)
        if io.get("dbg_bn") is not None:
            nc.sync.dma_start(out=io["dbg_bn"][:, 0:2], in_=stat[:])
            nc.sync.dma_start(out=io["dbg_bn"][:, 2:4], in_=gstat[:])
        mean = bn.tile([H, 1], F32, tag="mean")
        nc.vector.tensor_scalar(
            out=mean[:], in0=gstat[:, 0:1], scalar1=1.0 / U, scalar2=None, op0=AT.mult
        )
        ex2 = bn.tile([H, 1], F32, tag="ex2")
        nc.vector.tensor_scalar(
            out=ex2[:], in0=gstat[:, 1:2], scalar1=1.0 / U, scalar2=None, op0=AT.mult
        )
        m2 = bn.tile([H, 1], F32, tag="m2")
        nc.vector.tensor_tensor(out=m2[:], in0=mean[:], in1=mean[:], op=AT.mult)
        var = bn.tile([H, 1], F32, tag="var")
        nc.vector.tensor_tensor(out=var[:], in0=ex2[:], in1=m2[:], op=AT.subtract)
        vd = bn.tile([H, 1], F32, tag="vd")
        nc.vector.tensor_scalar(
            out=vd[:], in0=var[:], scalar1=1e-5, scalar2=None, op0=AT.add
        )
        rv = bn.tile([H, 1], F32, tag="rv")
        nc.vector.reciprocal(out=rv[:], in_=vd[:])
        rs = bn.tile([H, 1], F32, tag="rs")
        nc.scalar.activation(out=rs[:], in_=rv[:], func=AF.Sqrt)
        asc = bn.tile([H, 1], F32, tag="asc")
        nc.vector.tensor_tensor(out=asc[:], in0=rs[:], in1=gam_sb[:], op=AT.mult)
        mb = bn.tile([H, 1], F32, tag="mb")
        nc.vector.tensor_tensor(out=mb[:], in0=mean[:], in1=asc[:], op=AT.mult)
        bsh = bn.tile([H, 1], F32, tag="bsh")
        nc.vector.tensor_tensor(out=bsh[:], in0=bet_sb[:], in1=mb[:], op=AT.subtract)
        outr = out_ap.rearrange("(n p) h -> n p h", p=128)
        for n in range(NLOC):
            yt = bn.tile([H, 128], F32, tag="yt")
            nc.vector.tensor_scalar(
                out=yt[:],
                in0=node[:, n * 128 : (n + 1) * 128],
                scalar1=asc[:],
                scalar2=bsh[:],
                op0=AT.mult,
                op1=AT.add,
            )
            pt = bnp.tile([128, H], F32, tag="pt")
            nc.tensor.transpose(out=pt[:], in_=yt[:], identity=ident_sb[:])
            ot = bn.tile([128, H], F32, tag="ot")
            nc.scalar.activation(out=ot[:], in_=pt[:], func=AF.Copy)
            nc.sync.dma_start(out=outr[n], in_=ot[:])

    nodep_cm.__exit__(None, None, None)
    stack.close()


def make_nc(cfg):
    nc = bacc.Bacc(
        "TRN2",
        target_bir_lowering=False,
        debug=False,
        enable_asserts=False,
        num_devices=NCORES,
    )
    io = {}
    for name, shape, dt in INPUT_SPECS:
        io[name] = nc.dram_tensor(name, list(shape), dt, kind="ExternalInput").ap()
    for name, key in (
        ("uu_gidx", I32),
        ("uu_colw", F32),
        ("uu_disidx", I32),
    ):
        io[name] = nc.dram_tensor(
            name, [128, cfg["NBLK_u"]], key, kind="ExternalInput"
        ).ap()
    for name, key in (("cu_gidx", I32), ("cu_colw", F32)):
        io[name] = nc.dram_tensor(
            name, [128, cfg["NBLK_c"]], key, kind="ExternalInput"
        ).ap()
    io["loc_idx"] = nc.dram_tensor(
        "loc_idx", [128, NLOC], I32, kind="ExternalInput"
    ).ap()
    out_ap = nc.dram_tensor("out_shard", [LP, H], F32, kind="ExternalOutput").ap()
    if cfg.get("dbg"):
        io["dbg_dis"] = nc.dram_tensor("dbg_dis", [NCORES * LP], F32, kind="ExternalOutput").ap()
        io["dbg_disrow"] = nc.dram_tensor("dbg_disrow", [128, cfg["NBLK_u"]], F32, kind="ExternalOutput").ap()
        io["dbg_aggu"] = nc.dram_tensor("dbg_aggu", [ED, 512], F32, kind="ExternalOutput").ap()
        io["dbg_aggc"] = nc.dram_tensor("dbg_aggc", [DC, 512], F32, kind="ExternalOutput").ap()
        io["dbg_bn"] = nc.dram_tensor("dbg_bn", [H, 4], F32, kind="ExternalOutput").ap()
        io["dbg_ufa"] = nc.dram_tensor("dbg_ufa", [512, ED], F32, kind="ExternalOutput").ap()
        io["dbg_ufb"] = nc.dram_tensor("dbg_ufb", [512, ED], F32, kind="ExternalOutput").ap()
    with tile.TileContext(nc) as tc:
        build(nc, tc, io, out_ap, cfg)
    nc.compile()
    return nc


def kernel(**inputs):
    percore, cfg = host_prep(inputs)
    nc = make_nc(cfg)
    res = bass_utils.run_bass_kernel_spmd(nc, percore, core_ids=list(range(NCORES)))
    out = np.concatenate([res.results[k]["out_shard"][:L] for k in range(NCORES)], axis=0)
    return out.astype(np.float32)



# revision 2
# speedup vs baseline: 2.0927x; 2.0927x over previous
"""Trainium2 Bass kernel v2 for the 2-hop GNN (GCN + SAGE + BatchNorm).

Strategy (8 NeuronCores, SPMD, destination sharding):
  - Core k owns output rows [k*12500, (k+1)*12500); padded to 12544 = 98
    windows of 128 destinations.
  - Host prep is pure indexing / integer work: assemble userF by embedding
    lookups+concat (gathers, no arithmetic), bincount degrees/counts, sort
    and bucket edges by (dest window, source table class), pad to 128-edge
    blocks. All FP arithmetic runs on device.
  - Edge features are fetched with a handful of big dma_gather calls
    (int16 indices => tables split into <32k-row classes; 256B bf16 rows).
  - segment_sum via one-hot matmuls: per 128-edge block, S[e,d] =
    (col[e]==d) * coef[e] where coef folds the GCN norm dis[row]*dis[col]
    (resp. 1/cnt[dst] for SAGE mean) computed on device from uploaded
    integer degree values. PSUM accumulates per (window, class); DVE adds
    into bf16 agg_u [85, 12544] / agg_c [64, 12544] resident in SBUF.
  - Final: per 512-dest tile, ufT via DMA-transpose of a host-uploaded
    local userF slice, 3 bf16 matmuls per hop + leaky relu; BN stats
    AllReduced across cores.
"""

import numpy as np
import ml_dtypes

import concourse.bass as bass
import concourse.bacc as bacc
import concourse.tile as tile
import concourse.mybir as mybir
from concourse import bass_utils

F32 = mybir.dt.float32
BF16 = mybir.dt.bfloat16
I16 = mybir.dt.int16

U1 = 50000
U2 = 50000
U = 100000
C = 200000
E = 1000000
ED = 85
DC = 64
H = 128
NH = 2

NCORES = 8
L = 12500
WIN = 128
NW = 98
LP = NW * WIN          # 12544
CS = 32000             # table class size (int16-indexable)
NCLS_U = 4             # ceil(100000/32000)
NCLS_C = 7             # ceil(200000/32000)
SUPERS = [list(range(0, 33)), list(range(33, 66)), list(range(66, 98))]
FTILE = 512
NT = (LP + FTILE - 1) // FTILE   # 25
BF = ml_dtypes.bfloat16


def _bucket_rel(row, col, ncls, aux):
    """Bucket edges by (dest core, dest window, source class); pad to 128.

    Returns per-core slot arrays (colw/idx16/aux) + shared block metadata.
    """
    ne = len(row)
    shard = col // L
    lc = col % L
    w = lc // WIN
    cw = (lc % WIN).astype(np.float32)
    cls = row // CS
    bid = ((shard * NW) + w) * ncls + cls
    nbuck = NCORES * NW * ncls
    counts = np.bincount(bid, minlength=nbuck)
    starts = np.zeros(nbuck, np.int64)
    np.cumsum(counts[:-1], out=starts[1:])
    order = np.argsort(bid, kind="stable")
    rank = np.empty(ne, np.int64)
    rank[order] = np.arange(ne) - starts[bid[order]]
    bmat = np.ceil(counts.reshape(NCORES, NW, ncls).max(axis=0) / 128.0).astype(np.int64)

    blockbase = np.zeros((NW, ncls), np.int64)
    callinfo = []
    nblk = 0
    for s, wins in enumerate(SUPERS):
        for r in range(ncls):
            cb = nblk
            js = []
            for w_ in wins:
                b = int(bmat[w_, r])
                if b == 0:
                    continue
                blockbase[w_, r] = nblk
                js.append((w_, b))
                nblk += b
            callinfo.append({"s": s, "r": r, "base": cb, "nblk": nblk - cb, "wins": js})
    NBLK = nblk

    j = blockbase[w, cls] + rank // 128
    p = rank % 128
    colw = np.full((NCORES, 128, NBLK), -1.0, np.float32)
    colw[shard, p, j] = cw
    idxg = np.zeros((NCORES, 128, NBLK), np.int16)
    idxg[shard, p, j] = (row - cls * CS).astype(np.int16)
    out = {"colw": colw}
    for name, vals in aux.items():
        a = np.zeros((NCORES, 128, NBLK), np.float32)
        a[shard, p, j] = vals.astype(np.float32)
        out[name] = a

    slabs = []
    off16 = 0
    for ci in callinfo:
        b0, nb = ci["base"], ci["nblk"]
        ci["off16"] = off16
        ci["n"] = nb * 128
        ci["n16"] = nb * 128 // 16
        if nb == 0:
            continue
        arr = idxg[:, :, b0 : b0 + nb]                              # [NC,128,nb]
        flat = arr.transpose(0, 2, 1).reshape(NCORES, nb * 128)     # chunk-major g
        wrap = flat.reshape(NCORES, nb * 8, 16).transpose(0, 2, 1)  # [NC,16,n/16]
        slabs.append(np.tile(wrap, (1, 8, 1)))
        off16 += ci["n16"]
    idx16 = (np.concatenate(slabs, axis=2) if slabs
             else np.zeros((NCORES, 128, 0), np.int16)).astype(np.int16)
    out["idx16"] = idx16
    meta = {"callinfo": callinfo, "NBLK": NBLK, "TOT16": off16,
            "first_r": {}, "last_r": {}}
    for w_ in range(NW):
        rs = [r for r in range(ncls) if bmat[w_, r] > 0]
        if rs:
            meta["first_r"][w_] = rs[0]
            meta["last_r"][w_] = rs[-1]
    return out, meta


def host_prep(inputs):
    uf = np.asarray(inputs["u_feature"], dtype=np.float32)
    emb = np.asarray(inputs["emb_table"], dtype=np.float32)
    no_N = np.asarray(inputs["no_Nidx"]).astype(np.int64)
    e_tabs = {c: np.asarray(inputs[f"e{c}"], dtype=np.float32) for c in (0, 3, 7, 8, 9)}
    newF = np.concatenate(
        [
            e_tabs[0][uf[:, 0].astype(np.int64)],
            uf[:, 1:3],
            e_tabs[3][uf[:, 3].astype(np.int64)],
            uf[:, 4:7],
            e_tabs[7][uf[:, 7].astype(np.int64)],
            e_tabs[8][uf[:, 8].astype(np.int64)],
            e_tabs[9][uf[:, 9].astype(np.int64)],
        ],
        axis=1,
    )
    userF = np.concatenate([newF, emb[no_N]], axis=0)   # [100000, 85]

    utab = np.zeros((U, 128), BF)
    utab[:, :ED] = userF.astype(BF)
    ctab = np.zeros((C, 128), BF)
    ctab[:, :DC] = np.asarray(inputs["comment_x"], dtype=np.float32).astype(BF)

    ufp = np.zeros((NCORES * L + (LP - L), ED), np.float32)
    ufp[:U] = userF
    ulocs = [np.ascontiguousarray(ufp[k * L : k * L + LP]).astype(BF)
             for k in range(NCORES)]

    edge_uu = np.asarray(inputs["edge_uu"]).astype(np.int64)
    cu_src = np.asarray(inputs["edge_cu_src"]).astype(np.int64)
    cu_dst = np.asarray(inputs["edge_cu_dst"]).astype(np.int64)
    deg = np.bincount(edge_uu[1], minlength=U)
    cnt = np.bincount(cu_dst, minlength=U)

    uu_arr, uu_meta = _bucket_rel(
        edge_uu[0], edge_uu[1], NCLS_U,
        {"wdeg": deg[edge_uu[0]], "cdeg": deg[edge_uu[1]]},
    )
    cu_arr, cu_meta = _bucket_rel(cu_src, cu_dst, NCLS_C, {"wcnt": cnt[cu_dst]})

    iota = np.tile(np.arange(WIN, dtype=np.float32), (128, 1))
    ident = np.eye(128, dtype=np.float32)

    shared = {
        "utab": utab,
        "ctab": ctab,
        "iota": iota,
        "ident": ident,
        "wg": np.asarray(inputs["gcn_w"], np.float32).astype(BF),
        "wr": np.asarray(inputs["sage_r_w"], np.float32).astype(BF),
        "wl": np.asarray(inputs["sage_l_w"], np.float32).astype(BF),
        "gcn_b": np.asarray(inputs["gcn_b"], np.float32),
        "sage_l_b": np.asarray(inputs["sage_l_b"], np.float32),
        "bn_gamma": np.asarray(inputs["bn_gamma"], np.float32),
        "bn_beta": np.asarray(inputs["bn_beta"], np.float32),
    }
    percore = []
    for k in range(NCORES):
        m = dict(shared)
        m["uloc"] = ulocs[k]
        m["colw_u"] = uu_arr["colw"][k]
        m["wdeg"] = uu_arr["wdeg"][k]
        m["cdeg"] = uu_arr["cdeg"][k]
        m["idx_u"] = uu_arr["idx16"][k]
        m["colw_c"] = cu_arr["colw"][k]
        m["wcnt"] = cu_arr["wcnt"][k]
        m["idx_c"] = cu_arr["idx16"][k]
        percore.append(m)
    cfg = {"uu": uu_meta, "cu": cu_meta}
    return percore, cfg


def build(nc, tc, io, out_ap, cfg):
    AT = mybir.AluOpType
    AF = mybir.ActivationFunctionType
    AX = mybir.AxisListType
    RG = [list(range(NCORES))]
    uu, cu = cfg["uu"], cfg["cu"]
    NBU, NBC = uu["NBLK"], cu["NBLK"]

    bn_in = nc.dram_tensor("bn_in_d", [H, 2], F32).ap()
    bn_out = nc.dram_tensor("bn_out_d", [H, 2], F32, addr_space="Shared").ap()

    import contextlib

    stack = contextlib.ExitStack()
    big = stack.enter_context(tc.tile_pool(name="big", bufs=1))
    iota_sb = big.tile([128, WIN], F32, tag="iota")
    ident_sb = big.tile([128, 128], F32, tag="ident")
    wg_sb = [big.tile([ED, H], BF16, name=f"wg{h}", tag=f"wg{h}") for h in range(NH)]
    wr_sb = [big.tile([ED, H], BF16, name=f"wr{h}", tag=f"wr{h}") for h in range(NH)]
    wl_sb = [big.tile([DC, H], BF16, name=f"wl{h}", tag=f"wl{h}") for h in range(NH)]
    bh_sb = [big.tile([H, 1], F32, name=f"bh{h}", tag=f"bh{h}") for h in range(NH)]
    nbh_sb = [big.tile([H, 1], F32, name=f"nbh{h}", tag=f"nbh{h}") for h in range(NH)]
    gam_sb = big.tile([H, 1], F32, tag="gam")
    bet_sb = big.tile([H, 1], F32, tag="bet")
    colw_u_sb = big.tile([128, NBU], F32, tag="colw_u")
    ec_u_sb = big.tile([128, NBU], F32, tag="ec_u")
    colw_c_sb = big.tile([128, NBC], F32, tag="colw_c")
    ci_c_sb = big.tile([128, NBC], F32, tag="ci_c")
    idx_u_sb = big.tile([128, max(uu["TOT16"], 8)], I16, tag="idx_u")
    idx_c_sb = big.tile([128, max(cu["TOT16"], 8)], I16, tag="idx_c")
    agg_u = big.tile([ED, LP], BF16, tag="agg_u")
    agg_c = big.tile([DC, LP], BF16, tag="agg_c")
    node = big.tile([H, LP], BF16, tag="node")
    s_part = big.tile([H, NT], F32, tag="s_part")
    sq_part = big.tile([H, NT], F32, tag="sq_part")

    nc.sync.dma_start(out=iota_sb[:], in_=io["iota"])
    nc.sync.dma_start(out=ident_sb[:], in_=io["ident"])
    for h in range(NH):
        nc.sync.dma_start(out=wg_sb[h][:], in_=io["wg"][h])
        nc.sync.dma_start(out=wr_sb[h][:], in_=io["wr"][h])
        nc.sync.dma_start(out=wl_sb[h][:], in_=io["wl"][h])
    nc.sync.dma_start(out=gam_sb[:], in_=io["bn_gamma"][:, None])
    nc.sync.dma_start(out=bet_sb[:], in_=io["bn_beta"][:, None])
    nc.sync.dma_start(out=colw_u_sb[:], in_=io["colw_u"])
    nc.sync.dma_start(out=colw_c_sb[:], in_=io["colw_c"])
    nc.sync.dma_start(out=idx_u_sb[:, : uu["TOT16"]], in_=io["idx_u"])
    nc.sync.dma_start(out=idx_c_sb[:, : cu["TOT16"]], in_=io["idx_c"])

    # ---- biases: bh = gcn_b + sage_l_b; nbh = -bh ----------------------
    with tc.tile_pool(name="bias", bufs=2) as bp:
        for h in range(NH):
            t1 = bp.tile([H, 1], F32, tag="t1")
            t2 = bp.tile([H, 1], F32, tag="t2")
            nc.sync.dma_start(out=t1[:], in_=io["gcn_b"][h][:, None])
            nc.sync.dma_start(out=t2[:], in_=io["sage_l_b"][h][:, None])
            nc.vector.tensor_tensor(out=bh_sb[h][:], in0=t1[:], in1=t2[:], op=AT.add)
            nc.vector.tensor_scalar(out=nbh_sb[h][:], in0=bh_sb[h][:],
                                    scalar1=-1.0, scalar2=None, op0=AT.mult)

    # ---- per-edge coefficients ----------------------------------------
    # ec_u = dis(wdeg)*dis(cdeg), dis(x) = (x>0) * rsqrt(max(x,1))
    with tc.tile_pool(name="coef", bufs=1) as cp:
        wdeg = cp.tile([128, NBU], F32, tag="wdeg")
        cdeg = cp.tile([128, NBU], F32, tag="cdeg")
        wcnt = cp.tile([128, NBC], F32, tag="wcnt")
        nc.sync.dma_start(out=wdeg[:], in_=io["wdeg"])
        nc.sync.dma_start(out=cdeg[:], in_=io["cdeg"])
        nc.sync.dma_start(out=wcnt[:], in_=io["wcnt"])
        d1 = cp.tile([128, NBU], F32, tag="d1")
        d2 = cp.tile([128, NBU], F32, tag="d2")
        for src, dst in ((wdeg, d1), (cdeg, d2)):
            mx = cp.tile([128, NBU], F32, tag="mx")
            nc.vector.tensor_scalar(out=mx[:], in0=src[:], scalar1=1.0,
                                    scalar2=None, op0=AT.max)
            rc = cp.tile([128, NBU], F32, tag="rc")
            nc.vector.reciprocal(out=rc[:], in_=mx[:])
            rs = cp.tile([128, NBU], F32, tag="rs")
            nc.scalar.activation(out=rs[:], in_=rc[:], func=AF.Sqrt)
            mk = cp.tile([128, NBU], F32, tag="mk")
            nc.vector.tensor_scalar(out=mk[:], in0=src[:], scalar1=0.0,
                                    scalar2=None, op0=AT.is_gt)
            nc.vector.tensor_tensor(out=dst[:], in0=rs[:], in1=mk[:], op=AT.mult)
        nc.vector.tensor_tensor(out=ec_u_sb[:], in0=d1[:], in1=d2[:], op=AT.mult)
        cmx = cp.tile([128, NBC], F32, tag="cmx")
        nc.vector.tensor_scalar(out=cmx[:], in0=wcnt[:], scalar1=1.0,
                                scalar2=None, op0=AT.max)
        nc.vector.reciprocal(out=ci_c_sb[:], in_=cmx[:])

    # ---- gather + one-hot matmul aggregation ---------------------------
    # dma_gather calls are capped at CALLBLK blocks (SWDGE ring capacity)
    # and round-robined over 4 SWDGE queues so descriptor generation of
    # call i+1 overlaps the drain of call i.
    CALLBLK = 8
    qctr = [0]

    def agg_pass(meta, idx_sb, tab_io, ncls, colw_sb, coef_sb, agg, rows,
                 gath, aggp, spool, memset_windows):
        for ci in meta["callinfo"]:
            if ci["nblk"] == 0:
                continue
            r = ci["r"]
            c0 = r * CS
            c1 = min(c0 + CS, tab_io.shape[0])
            # flat per-chunk window list + start/stop flags for this class-call
            wflat = []
            for w_, B in ci["wins"]:
                for b in range(B):
                    wflat.append((w_, b == 0, b == B - 1))
            pm_open = {}
            for sc0 in range(0, ci["nblk"], CALLBLK):
                nblk_sc = min(CALLBLK, ci["nblk"] - sc0)
                n = nblk_sc * 128
                o16 = ci["off16"] + sc0 * 8
                g = gath.tile([128, CALLBLK * 128], BF16, tag="gath")
                nc.gpsimd.dma_gather(
                    out_ap=g[:, : nblk_sc * 128].rearrange("p (c e) -> p c e", e=128),
                    in_ap=tab_io[c0:c1, :],
                    idxs_ap=idx_sb[:, o16 : o16 + nblk_sc * 8],
                    num_idxs=n,
                    num_idxs_reg=n,
                    elem_size=128,
                    queue_num=qctr[0] % 4,
                )
                qctr[0] += 1
                for lc in range(nblk_sc):
                    chunk = sc0 + lc
                    w_, first, last = wflat[chunk]
                    jg = ci["base"] + chunk
                    S = spool.tile([128, WIN], BF16, tag="S")
                    nc.vector.tensor_scalar(
                        out=S[:], in0=iota_sb[:],
                        scalar1=colw_sb[:, jg : jg + 1],
                        scalar2=coef_sb[:, jg : jg + 1],
                        op0=AT.is_equal, op1=AT.mult)
                    if first:
                        pm_open[w_] = aggp.tile([128, WIN], F32, tag="pm",
                                                name=f"pm_{r}_{w_}")
                    pm = pm_open[w_]
                    nc.tensor.matmul(
                        out=pm[:], lhsT=g[:, lc * 128 : (lc + 1) * 128],
                        rhs=S[:], start=first, stop=last)
                    if last:
                        sl = agg[:, w_ * WIN : (w_ + 1) * WIN]
                        if meta["first_r"][w_] == r:
                            nc.vector.tensor_copy(out=sl, in_=pm[:rows, :])
                        else:
                            nc.vector.tensor_tensor(out=sl, in0=sl,
                                                    in1=pm[:rows, :], op=AT.add)
                        del pm_open[w_]
        for w_ in memset_windows:
            nc.vector.memset(agg[:, w_ * WIN : (w_ + 1) * WIN], 0.0)

    with (
        tc.tile_pool(name="gath", bufs=6) as gath,
        tc.tile_pool(name="aggp", bufs=4, space="PSUM") as aggp,
        tc.tile_pool(name="spool", bufs=8) as spool,
    ):
        mw_u = [w_ for w_ in range(NW) if w_ not in uu["first_r"]]
        mw_c = [w_ for w_ in range(NW) if w_ not in cu["first_r"]]
        agg_pass(uu, idx_u_sb, io["utab"], NCLS_U, colw_u_sb, ec_u_sb,
                 agg_u, ED, gath, aggp, spool, mw_u)
        agg_pass(cu, idx_c_sb, io["ctab"], NCLS_C,
                 colw_c_sb, ci_c_sb, agg_c, DC, gath, aggp, spool, mw_c)

    # ---- final: matmuls + leaky relu + hop sum + BN stats --------------
    with (
        tc.tile_pool(name="fin", bufs=2) as fin,
        tc.tile_pool(name="finp", bufs=2, space="PSUM") as finp,
    ):
        for t in range(NT):
            t0 = t * FTILE
            tn = min(FTILE, LP - t0)
            ufT = fin.tile([ED, FTILE], BF16, tag="ufT")
            nc.sync.dma_start_transpose(out=ufT[:, :tn], in_=io["uloc"][t0 : t0 + tn, :])
            rel = []
            for h in range(NH):
                ph = finp.tile([H, FTILE], F32, tag="ph")
                nc.tensor.matmul(out=ph[:, :tn], lhsT=wg_sb[h][:],
                                 rhs=agg_u[:, t0 : t0 + tn], start=True, stop=False)
                nc.tensor.matmul(out=ph[:, :tn], lhsT=wr_sb[h][:],
                                 rhs=ufT[:, :tn], start=False, stop=False)
                nc.tensor.matmul(out=ph[:, :tn], lhsT=wl_sb[h][:],
                                 rhs=agg_c[:, t0 : t0 + tn], start=False, stop=True)
                rp = fin.tile([H, FTILE], F32, tag="rp")
                nc.scalar.activation(out=rp[:, :tn], in_=ph[:, :tn], func=AF.Relu,
                                     bias=bh_sb[h][:])
                rn = fin.tile([H, FTILE], F32, tag="rn")
                nc.scalar.activation(out=rn[:, :tn], in_=ph[:, :tn], func=AF.Relu,
                                     bias=nbh_sb[h][:], scale=-1.0)
                rel.append((rp, rn))
            a1 = fin.tile([H, FTILE], F32, tag="a1")
            nc.vector.tensor_tensor(out=a1[:, :tn], in0=rel[0][0][:, :tn],
                                    in1=rel[1][0][:, :tn], op=AT.add)
            a2 = fin.tile([H, FTILE], F32, tag="a2")
            nc.vector.tensor_tensor(out=a2[:, :tn], in0=rel[0][1][:, :tn],
                                    in1=rel[1][1][:, :tn], op=AT.add)
            a3 = fin.tile([H, FTILE], F32, tag="a3")
            nc.vector.tensor_scalar(out=a3[:, :tn], in0=a2[:, :tn], scalar1=-0.3,
                                    scalar2=None, op0=AT.mult)
            nc.vector.tensor_tensor(out=node[:, t0 : t0 + tn], in0=a1[:, :tn],
                                    in1=a3[:, :tn], op=AT.add)
            if t < NT - 1:
                nc.vector.tensor_reduce(out=s_part[:, t : t + 1],
                                        in_=node[:, t0 : t0 + tn], axis=AX.X, op=AT.add)
                sqs = fin.tile([H, FTILE], F32, tag="sqs")
                nc.scalar.activation(out=sqs[:, :tn], in_=node[:, t0 : t0 + tn],
                                     func=AF.Square, accum_out=sq_part[:, t : t + 1])
        nc.vector.memset(node[:, L:LP], 0.0)
        t = NT - 1
        t0 = t * FTILE
        tn = LP - t0
        nc.vector.tensor_reduce(out=s_part[:, t : t + 1], in_=node[:, t0 : t0 + tn],
                                axis=AX.X, op=AT.add)
        sqs = fin.tile([H, FTILE], F32, tag="sqs")
        nc.scalar.activation(out=sqs[:, :tn], in_=node[:, t0 : t0 + tn],
                             func=AF.Square, accum_out=sq_part[:, t : t + 1])

    # ---- BN: allreduce stats, normalize, transpose out -----------------
    with (
        tc.tile_pool(name="bn", bufs=2) as bn,
        tc.tile_pool(name="bnp", bufs=2, space="PSUM") as bnp,
    ):
        stat = bn.tile([H, 2], F32, tag="stat")
        nc.vector.tensor_reduce(out=stat[:, 0:1], in_=s_part[:], axis=AX.X, op=AT.add)
        nc.vector.tensor_reduce(out=stat[:, 1:2], in_=sq_part[:], axis=AX.X, op=AT.add)
        nc.sync.dma_start(out=bn_in, in_=stat[:])
        nc.gpsimd.collective_compute(
            "AllReduce", mybir.AluOpType.add, replica_groups=RG,
            ins=[bn_in], outs=[bn_out])
        gstat = bn.tile([H, 2], F32, tag="gstat")
        nc.sync.dma_start(out=gstat[:], in_=bn_out)
        mean = bn.tile([H, 1], F32, tag="mean")
        nc.vector.tensor_scalar(out=mean[:], in0=gstat[:, 0:1], scalar1=1.0 / U,
                                scalar2=None, op0=AT.mult)
        ex2 = bn.tile([H, 1], F32, tag="ex2")
        nc.vector.tensor_scalar(out=ex2[:], in0=gstat[:, 1:2], scalar1=1.0 / U,
                                scalar2=None, op0=AT.mult)
        m2 = bn.tile([H, 1], F32, tag="m2")
        nc.vector.tensor_tensor(out=m2[:], in0=mean[:], in1=mean[:], op=AT.mult)
        var = bn.tile([H, 1], F32, tag="var")
        nc.vector.tensor_tensor(out=var[:], in0=ex2[:], in1=m2[:], op=AT.subtract)
        vd = bn.tile([H, 1], F32, tag="vd")
        nc.vector.tensor_scalar(out=vd[:], in0=var[:], scalar1=1e-5, scalar2=None,
                                op0=AT.add)
        rv = bn.tile([H, 1], F32, tag="rv")
        nc.vector.reciprocal(out=rv[:], in_=vd[:])
        rs = bn.tile([H, 1], F32, tag="rs")
        nc.scalar.activation(out=rs[:], in_=rv[:], func=AF.Sqrt)
        asc = bn.tile([H, 1], F32, tag="asc")
        nc.vector.tensor_tensor(out=asc[:], in0=rs[:], in1=gam_sb[:], op=AT.mult)
        mb = bn.tile([H, 1], F32, tag="mb")
        nc.vector.tensor_tensor(out=mb[:], in0=mean[:], in1=asc[:], op=AT.mult)
        bsh = bn.tile([H, 1], F32, tag="bsh")
        nc.vector.tensor_tensor(out=bsh[:], in0=bet_sb[:], in1=mb[:], op=AT.subtract)
        outv = out_ap.rearrange("(n p) h -> p n h", p=128)   # [128, 98, H]
        for n0 in range(0, NW, 4):
            gn = min(4, NW - n0)
            stg = bn.tile([128, 4 * H], F32, tag="stg")
            for gi in range(gn):
                n = n0 + gi
                yt = bn.tile([H, 128], F32, tag="yt")
                nc.vector.tensor_scalar(
                    out=yt[:], in0=node[:, n * 128 : (n + 1) * 128],
                    scalar1=asc[:], scalar2=bsh[:], op0=AT.mult, op1=AT.add)
                pt = bnp.tile([128, H], F32, tag="pt")
                nc.tensor.transpose(out=pt[:], in_=yt[:], identity=ident_sb[:])
                nc.scalar.activation(out=stg[:, gi * H : (gi + 1) * H], in_=pt[:],
                                     func=AF.Copy)
            nc.sync.dma_start(
                out=outv[:, n0 : n0 + gn, :],
                in_=stg[:, : gn * H].rearrange("p (g h) -> p g h", h=H))

    stack.close()


def make_nc(cfg):
    uu, cu = cfg["uu"], cfg["cu"]
    nc = bacc.Bacc(
        "TRN2",
        target_bir_lowering=False,
        debug=False,
        enable_asserts=False,
        num_devices=NCORES,
        num_swdge_queues=4,
    )
    io = {}
    specs = [
        ("utab", (U, 128), BF16),
        ("ctab", (C, 128), BF16),
        ("uloc", (LP, ED), BF16),
        ("iota", (128, WIN), F32),
        ("ident", (128, 128), F32),
        ("wg", (NH, ED, H), BF16),
        ("wr", (NH, ED, H), BF16),
        ("wl", (NH, DC, H), BF16),
        ("gcn_b", (NH, H), F32),
        ("sage_l_b", (NH, H), F32),
        ("bn_gamma", (H,), F32),
        ("bn_beta", (H,), F32),
        ("colw_u", (128, uu["NBLK"]), F32),
        ("wdeg", (128, uu["NBLK"]), F32),
        ("cdeg", (128, uu["NBLK"]), F32),
        ("idx_u", (128, uu["TOT16"]), I16),
        ("colw_c", (128, cu["NBLK"]), F32),
        ("wcnt", (128, cu["NBLK"]), F32),
        ("idx_c", (128, cu["TOT16"]), I16),
    ]
    for name, shape, dt in specs:
        io[name] = nc.dram_tensor(name, list(shape), dt, kind="ExternalInput").ap()
    out_ap = nc.dram_tensor("out_shard", [LP, H], F32, kind="ExternalOutput").ap()
    with tile.TileContext(nc) as tc:
        build(nc, tc, io, out_ap, cfg)
    nc.compile()
    return nc


def kernel(**inputs):
    percore, cfg = host_prep(inputs)
    nc = make_nc(cfg)
    res = bass_utils.run_bass_kernel_spmd(nc, percore, core_ids=list(range(NCORES)))
    out = np.concatenate([res.results[k]["out_shard"][:L] for k in range(NCORES)], axis=0)
    return out.astype(np.float32)
